# revision 20
# baseline (speedup 1.0000x reference)
"""TRN2 Bass kernel for nn_AttentionBlock (N=4, C=256, L=4096, 4 heads, AGGR=4).

Sharding: 8 cores = (batch n, L-half). Core c handles n=c//2, query positions
l in [half*2048, (half+1)*2048). Each core computes k/v from the full
aggregated sequence of its batch (L2=1024) and produces the full output slice
out[n][:, l_half] -- no cross-core reduction needed.

The host hands each core x[n] with columns PERMUTED so the core's own query
half comes first (attention is permutation-invariant over key positions, and
the 4-wide pooling windows stay intact), so the query slice is a static
[:, 0:2048] view and is available as soon as the first DMA half lands.

Cost-model shape: every engine instruction costs (free-dim cols) x cycle_t;
PE matmuls cost (out free cols) x 0.417ns regardless of contraction width.
The kernel is PE-bound (~70us of matmul cols), so softmax exp -- the other
big consumer (64 tiles x 1024 cols) -- is split across three engines so none
exceeds PE: ACT runs real Exp; Pool and DVE run a Schraudolph bit-trick exp
(i32 = trunc(S*2^23*log2e*0.125 + bias), bitcast as f32 ~ exp(S/8) within
3%), writing int32 tiles the o-matmul consumes as float32r. f32->f32r
bitcast views avoid all weight/x re-typing copies.
"""

import numpy as np

N, C, L = 4, 256, 4096
HEAD_DIM = 64
H = C // HEAD_DIM          # 4 heads
AGGR = 4
L2 = L // AGGR             # 1024 aggregated positions
LH = L // 2                # 2048 query positions per core
BN_EPS = 1e-5
N_CORES = 8

# Schraudolph exp-trick constants, int16/bfloat16 variant (trunc/floor):
# i16 = trunc(S * 2^7*log2e/8 + (127*2^7 - bias)); i16 bits read as bf16
# give exp(S/8) within ~3%. Folds the 1/sqrt(E)=1/8 score scale.
TRICK_A = 128.0 * 1.4426950408889634 * 0.125
TRICK_B = float(127 << 7) - 366400.0 / 65536.0

_CACHE = {}


def _build_program():
    import concourse.bass as bass
    import concourse.bacc as bacc
    import concourse.tile as tile
    from concourse import mybir
    from contextlib import ExitStack

    dt = mybir.dt
    f32 = dt.float32
    f32r = dt.float32r
    bf16 = dt.bfloat16
    i16 = dt.int16
    AF = mybir.ActivationFunctionType
    Alu = mybir.AluOpType

    nc = bacc.Bacc("TRN2", debug=False, num_devices=N_CORES)

    xf_d = nc.dram_tensor("x_full", [C, L], f32, kind="ExternalInput")
    wqt_d = nc.dram_tensor("wqt", [C, C], f32, kind="ExternalInput")
    wkt_d = nc.dram_tensor("wkt", [C, C], f32, kind="ExternalInput")
    wvt_d = nc.dram_tensor("wvt", [C, C], f32, kind="ExternalInput")
    wot_d = nc.dram_tensor("wot", [C, C], f32, kind="ExternalInput")
    wat_d = nc.dram_tensor("wat", [C, C], f32, kind="ExternalInput")
    # rows: bq, bk, t(bn-folded xa bias), bo
    bp_d = nc.dram_tensor("biasp", [4, C], f32, kind="ExternalInput")
    bv_d = nc.dram_tensor("bv", [C], f32, kind="ExternalInput")
    out_d = nc.dram_tensor("out", [C, LH], f32, kind="ExternalOutput")

    # exp engine per (iter_index, mt): A=ACT real exp, D=DVE int-trick exp
    # (gpsimd can't read PSUM, so only ACT/DVE can consume the S tiles)
    EXP_ENG = {}
    for it in range(8):
        for mt in range(8):
            d = (mt == 7) or (mt == 6 and it < 6)
            EXP_ENG[(it, mt)] = 'D' if d else 'A'

    with tile.TileContext(nc) as tc, ExitStack() as ctx:
        pp = ctx.enter_context(tc.tile_pool(name="persist", bufs=1))
        scr_w = ctx.enter_context(tc.tile_pool(name="scr_w", bufs=1))
        scr_p = ctx.enter_context(tc.tile_pool(name="scr_p", bufs=3))
        at_pool = ctx.enter_context(tc.tile_pool(name="at", bufs=6))
        ai_pool = ctx.enter_context(tc.tile_pool(name="ai", bufs=5))
        oa_pool = ctx.enter_context(tc.tile_pool(name="oa", bufs=2))
        outp = ctx.enter_context(tc.tile_pool(name="outp", bufs=3))
        r_pool = ctx.enter_context(tc.tile_pool(name="rp", bufs=2))
        R_pool = ctx.enter_context(tc.tile_pool(name="Rp", bufs=3))

        ps_s = ctx.enter_context(tc.tile_pool(name="ps_s", bufs=3, space="PSUM"))
        ps_o = ctx.enter_context(tc.tile_pool(name="ps_o", bufs=2, space="PSUM"))

        # ---- persistent tiles ----
        xf = [pp.tile([128, L], f32, name=f"xf{ct}", tag=f"xf{ct}")
              for ct in range(2)]
        # query-half of x re-DMA'd as f32r for the q projection (the BIR
        # verifier requires f32r matmul inputs to be produced as f32r)
        xq_r = [pp.tile([128, LH], f32r, name=f"xqr{ct}", tag=f"xqr{ct}")
                for ct in range(2)]
        q_r = [pp.tile([128, LH], bf16, name=f"qr{ct}", tag=f"qr{ct}")
               for ct in range(2)]
        k_r = [pp.tile([128, L2], bf16, name=f"kr{ct}", tag=f"kr{ct}")
               for ct in range(2)]
        xa_r = [pp.tile([128, L2], f32r, name=f"xar{ct}", tag=f"xar{ct}")
                for ct in range(2)]
        p_r = [pp.tile([128, L2], f32r, name=f"pr{ct}", tag=f"pr{ct}")
               for ct in range(2)]
        # v'^T per m-tile: 4 heads x (64 cols + ones col), bf16
        v_r = [pp.tile([128, 4 * 65], bf16, name=f"vr{mt}", tag=f"vr{mt}")
               for mt in range(8)]
        bias_t = [pp.tile([128, 4], f32, name=f"bias{ct}", tag=f"bias{ct}")
                  for ct in range(2)]
        bvb = pp.tile([128, C], f32, name="bvb", tag="bvb")

        # ---- DMAs: x on the SP queue; weights/biases on the ACT queue so
        # they land in parallel with the 4MB of x ----
        for half in range(2):
            for ct in range(2):
                for sub in range(2):
                    c0 = half * 2048 + sub * 1024
                    nc.sync.dma_start(
                        xf[ct][:, c0:c0 + 1024],
                        xf_d.ap()[ct * 128:(ct + 1) * 128, c0:c0 + 1024])
        wt_dram = {"wqt": wqt_d, "wkt": wkt_d, "wvt": wvt_d, "wot": wot_d,
                   "wat": wat_d}
        w_f = {}
        for wname in ("wat", "wkt", "wqt", "wvt", "wot"):
            wf = scr_w.tile([128, 512], f32r, name=f"wf_{wname}",
                            tag=f"wf_{wname}")
            src = wt_dram[wname].ap().bitcast(f32r).rearrange(
                "(k p) o -> p k o", p=128)
            nc.scalar.dma_start(wf[:].rearrange("p (k o) -> p k o", k=2), src)
            w_f[wname] = wf
        # xq chunks on the ACT queue, first 512 cols first (q_chunk(0) gate)
        for c0, cw in ((0, 512), (512, 1536)):
            for ct in range(2):
                nc.scalar.dma_start(
                    xq_r[ct][:, c0:c0 + cw],
                    xf_d.ap().bitcast(f32r)[ct * 128:(ct + 1) * 128,
                                            c0:c0 + cw])
        for ct in range(2):
            nc.scalar.dma_start(
                bias_t[ct][:], bp_d.ap().rearrange("b (k p) -> k p b", p=128)[ct])
        bv_f = r_pool.tile([1, C], f32, name="bv_f", tag="bv_f", bufs=1)
        nc.scalar.dma_start(bv_f[:], bv_d.ap().rearrange("(a o) -> a o", a=1))

        # ---- constants ----
        for mt in range(8):
            nc.gpsimd.memset(
                v_r[mt][:].rearrange("p (h e) -> p h e", e=65)[:, :, 64], 1.0)
        # pre-warm the ACT exp table during the idle prefix
        warm = scr_w.tile([1, 8], f32, name="warm", tag="warm")
        ones_f = scr_w.tile([1, 8], f32, name="ones_f", tag="ones_f")
        nc.gpsimd.memset(ones_f[:], 1.0)
        nc.scalar.activation(warm[:], ones_f[:], AF.Exp, scale=1.0)
        # bv broadcast to all partitions (for the v-drain fused bias)
        nc.gpsimd.partition_broadcast(bvb[:], bv_f[:], channels=128)

        def w_block(wname, cch, ct_out):
            # lhsT block [c_in 128, c_out 128] for chunk cch, out tile ct_out
            return w_f[wname][:, cch * 256 + ct_out * 128:
                              cch * 256 + ct_out * 128 + 128]

        # ---- pool quadrants: p = avg4 + max4 ----
        def pool_quadrant(mc, ct, eng, sub=None):
            c0, cw = mc * 2048, 2048
            s0, sw = mc * 512, 512
            if sub is not None:
                c0, cw = c0 + sub * 1024, 1024
                s0, sw = s0 + sub * 256, 256
            xv = xf[ct][:, c0:c0 + cw].rearrange("p (m g) -> p m g", g=4)
            a1 = scr_p.tile([128, 512], f32, name="pa1", tag="pa1")
            a2 = scr_p.tile([128, 512], f32, name="pa2", tag="pa2")
            m1 = scr_p.tile([128, 512], f32, name="pm1", tag="pm1")
            m2 = scr_p.tile([128, 512], f32, name="pm2", tag="pm2")
            eng.tensor_tensor(a1[:, 0:sw], xv[:, :, 0], xv[:, :, 1], Alu.add)
            eng.tensor_tensor(a2[:, 0:sw], xv[:, :, 2], xv[:, :, 3], Alu.add)
            eng.tensor_tensor(m1[:, 0:sw], xv[:, :, 0], xv[:, :, 1], Alu.max)
            eng.tensor_tensor(m2[:, 0:sw], xv[:, :, 2], xv[:, :, 3], Alu.max)
            eng.tensor_tensor(a1[:, 0:sw], a1[:, 0:sw], a2[:, 0:sw], Alu.add)
            eng.tensor_tensor(m1[:, 0:sw], m1[:, 0:sw], m2[:, 0:sw], Alu.max)
            eng.scalar_tensor_tensor(
                p_r[ct][:, s0:s0 + sw], a1[:, 0:sw], 0.25, m1[:, 0:sw],
                Alu.mult, Alu.add)

        # first quadrants in halves: start right after the first x chunks
        pool_quadrant(0, 0, nc.vector, sub=0)
        pool_quadrant(0, 0, nc.vector, sub=1)
        pool_quadrant(0, 1, nc.vector, sub=0)
        pool_quadrant(0, 1, nc.vector, sub=1)

        # ---- projection chunk helpers ----
        def proj_chunk(wname, src, dst, bias_col, nn2, eng):
            for ct_out in range(2):
                ps = ps_s.tile([128, 512], f32, name="ps_s", tag="ps_s")
                for cch in range(2):
                    nc.tensor.matmul(
                        ps[:], w_block(wname, cch, ct_out),
                        src[cch][:, nn2 * 512:(nn2 + 1) * 512],
                        start=(cch == 0), stop=(cch == 1))
                if eng is nc.scalar:
                    nc.scalar.add(dst[ct_out][:, nn2 * 512:(nn2 + 1) * 512],
                                  ps[:], bias_t[ct_out][:, bias_col:bias_col + 1])
                else:
                    eng.tensor_scalar(
                        dst[ct_out][:, nn2 * 512:(nn2 + 1) * 512], ps[:],
                        bias_t[ct_out][:, bias_col:bias_col + 1], None, Alu.add)

        def q_chunk(lcq, eng):
            for ct_out in range(2):
                ps = ps_s.tile([128, 512], f32, name="ps_s", tag="ps_s")
                for cch in range(2):
                    nc.tensor.matmul(
                        ps[:], w_block("wqt", cch, ct_out),
                        xq_r[cch][:, lcq * 512:(lcq + 1) * 512],
                        start=(cch == 0), stop=(cch == 1))
                if eng is nc.scalar:
                    nc.scalar.add(q_r[ct_out][:, lcq * 512:(lcq + 1) * 512],
                                  ps[:], bias_t[ct_out][:, 0:1])
                else:
                    eng.tensor_scalar(
                        q_r[ct_out][:, lcq * 512:(lcq + 1) * 512], ps[:],
                        bias_t[ct_out][:, 0:1], None, Alu.add)

        def v_block(mt, drain_eng, vpool=None):
            vpool = vpool or ps_o
            tag = "ps_o" if vpool is ps_o else "ps_s"
            pv = vpool.tile([128, C], f32, name="ps_v", tag=tag)
            for cch in range(2):
                nc.tensor.matmul(
                    pv[:], xa_r[cch][:, mt * 128:(mt + 1) * 128],
                    w_f["wvt"][:, cch * 256:(cch + 1) * 256],
                    start=(cch == 0), stop=(cch == 1))
            vv = v_r[mt][:].rearrange("p (h e) -> p h e", e=65)
            # fused +bv via the broadcast bias tile
            drain_eng.scalar_tensor_tensor(
                vv[:, :, 0:64], pv[:].rearrange("p (h e) -> p h e", e=64),
                1.0, bvb[:].rearrange("p (h e) -> p h e", e=64),
                Alu.mult, Alu.add)

        # ---- prefix: q lc0, xa/k chunk n0, rest of pool, v 0-3 ----
        q_chunk(0, nc.scalar)
        proj_chunk("wat", p_r, xa_r, 2, 0, nc.scalar)
        proj_chunk("wkt", xa_r, k_r, 1, 0, nc.scalar)
        pool_quadrant(1, 0, nc.vector)
        for mt in range(2):
            v_block(mt, nc.vector)
        pool_quadrant(1, 1, nc.vector)
        for mt in range(2, 4):
            v_block(mt, nc.vector)

        # ---- attention: o-matmuls lag exp by one m-tile; the previous
        # iteration's softmax-normalize and Wo conv are emitted inside the
        # next iteration's S/exp stream so they overlap it ----
        oa_tiles = {}

        def norm_prev(state):
            lc, hp, po = state
            oa = oa_tiles[lc]
            for h2 in range(2):
                r_t = r_pool.tile([1, 512], f32, name="r", tag="r")
                nc.vector.reciprocal(r_t[:], po[h2][64:65, :])
                R_t = R_pool.tile([64, 512], f32, name="R", tag="R")
                nc.gpsimd.partition_broadcast(R_t[:], r_t[:], channels=64)
                nc.vector.tensor_tensor(
                    oa[hp][h2 * 64:(h2 + 1) * 64, :], po[h2][0:64, :],
                    R_t[:], Alu.mult)

        def wo_prev(state):
            lc, hp, po = state
            if hp != 1:
                return
            oa = oa_tiles[lc]
            for ct_out in range(2):
                psW = ps_s.tile([128, 512], f32, name="ps_s", tag="ps_s")
                for cch in range(2):
                    nc.tensor.matmul(
                        psW[:], w_block("wot", cch, ct_out), oa[cch][:],
                        start=(cch == 0), stop=(cch == 1))
                out_t = outp.tile([128, 512], f32, name="out", tag="out")
                nc.scalar.add(out_t[:], psW[:], bias_t[ct_out][:, 3:4])
                nc.sync.dma_start(
                    out_d.ap()[ct_out * 128:(ct_out + 1) * 128,
                               lc * 512:(lc + 1) * 512], out_t[:])
            del oa_tiles[lc]

        # pending o-matmul FIFO: one pair popped per (S, exp) step, crossing
        # iteration boundaries so PE never waits on the last exp of an iter
        pending = []
        it_idx = [0]

        def emit_iter(lc, hp, prev_state, mid_hook=None):
            it = it_idx[0]
            it_idx[0] += 1
            if hp == 0:
                oa_tiles[lc] = [
                    oa_pool.tile([128, 512], f32r, name=f"oa{ct}",
                                 tag=f"oa{ct}") for ct in range(2)]
            po = [ps_o.tile([65, 512], f32, name="ps_o", tag="ps_o")
                  for _ in range(2)]

            def make_o(mt, at_ap):
                def emit():
                    for h2 in range(2):
                        h = 2 * hp + h2
                        nc.tensor.matmul(
                            po[h2][:], v_r[mt][:, h * 65:h * 65 + 65],
                            at_ap[:, h2 * 512:(h2 + 1) * 512],
                            start=(mt == 0), stop=(mt == 7))
                return emit

            for mt in range(8):
                if mt == 4 and mid_hook is not None:
                    mid_hook()
                ps = ps_s.tile([128, L2], f32, name="ps_s", tag="ps_s")
                for h2 in range(2):
                    nc.tensor.matmul(
                        ps[:, h2 * 512:(h2 + 1) * 512],
                        k_r[hp][h2 * 64:(h2 + 1) * 64, mt * 128:(mt + 1) * 128],
                        q_r[hp][h2 * 64:(h2 + 1) * 64, lc * 512:(lc + 1) * 512],
                        start=True, stop=True)
                eng = EXP_ENG[(it, mt)]
                if eng == 'A':
                    at = at_pool.tile([128, 1024], bf16, name="at", tag="at")
                    nc.scalar.activation(at[:], ps[:], AF.Exp, scale=0.125)
                    at_ap = at[:]
                else:
                    ai = ai_pool.tile([128, 1024], i16, name="ai", tag="ai")
                    nc.vector.tensor_scalar(ai[:], ps[:], TRICK_A, TRICK_B,
                                            Alu.mult, Alu.add)
                    at_ap = ai[:].bitcast(bf16)
                pending.append(make_o(mt, at_ap))
                # with the deeper o-FIFO, the previous iteration's last
                # o-matmul is popped during step mt1, so its normalize may
                # be emitted no earlier than mt2 (else it misses mt7)
                if mt == 2 and prev_state is not None:
                    norm_prev(prev_state)
                if mt == 5 and prev_state is not None:
                    wo_prev(prev_state)
                if len(pending) >= 3:
                    pending.pop(0)()
            return (lc, hp, po)

        # iteration (0,0) with the n1 projections + v 4-7 emitted mid-stream
        def mid():
            proj_chunk("wat", p_r, xa_r, 2, 1, nc.scalar)
            proj_chunk("wkt", xa_r, k_r, 1, 1, nc.scalar)
            for mt in range(4, 8):
                v_block(mt, nc.vector, vpool=ps_s)

        state = emit_iter(0, 0, None, mid_hook=mid)
        q_after = {(0, 1): 1, (1, 0): 2, (1, 1): 3}
        for lc, hp in [(0, 1), (1, 0), (1, 1), (2, 0), (2, 1), (3, 0), (3, 1)]:
            state = emit_iter(lc, hp, state)
            lcq = q_after.get((lc, hp))
            if lcq:
                q_chunk(lcq, nc.vector)
        while pending:
            pending.pop(0)()
        norm_prev(state)
        wo_prev(state)

    nc.compile()
    return nc


def _get_program():
    if "nc" not in _CACHE:
        _CACHE["nc"] = _build_program()
    return _CACHE["nc"]


def kernel(x, Wq, bq, Wk, bk, Wv, bv, Wo, bo, Wa,
           g1, b1, m1, v1, g2, b2, m2, v2):
    from concourse import bass_utils

    nc = _get_program()

    x = np.asarray(x, dtype=np.float32)
    # fold both eval-mode BNs into a per-channel affine: xa = s*(Wa@p) + t
    s1 = np.asarray(g1) / np.sqrt(np.asarray(v1) + BN_EPS)
    t1 = np.asarray(b1) - np.asarray(m1) * s1
    s2 = np.asarray(g2) / np.sqrt(np.asarray(v2) + BN_EPS)
    t2 = np.asarray(b2) - np.asarray(m2) * s2
    s = (s1 * s2).astype(np.float32)
    t = (t1 * s2 + t2).astype(np.float32)

    wat = (np.asarray(Wa) * s[:, None]).astype(np.float32).T.copy()
    wqt = np.asarray(Wq, dtype=np.float32).T.copy()
    wkt = np.asarray(Wk, dtype=np.float32).T.copy()
    wvt = np.asarray(Wv, dtype=np.float32).T.copy()
    wot = np.asarray(Wo, dtype=np.float32).T.copy()
    biasp = np.stack([np.asarray(bq), np.asarray(bk), t,
                      np.asarray(bo)]).astype(np.float32)
    bvv = np.asarray(bv, dtype=np.float32)

    shared = {"wqt": wqt, "wkt": wkt, "wvt": wvt, "wot": wot, "wat": wat,
              "biasp": biasp, "bv": bvv}
    in_maps = []
    for c in range(N_CORES):
        n, half = c // 2, c % 2
        m = dict(shared)
        xs = x[n]
        if half == 0:
            m["x_full"] = np.ascontiguousarray(xs)
        else:
            # core's own query half first; key order is irrelevant
            # (pool windows intact, attention permutation-invariant)
            m["x_full"] = np.concatenate([xs[:, LH:], xs[:, :LH]], axis=1)
        in_maps.append(m)

    res = bass_utils.run_bass_kernel_spmd(nc, in_maps,
                                          core_ids=list(range(N_CORES)))
    out = np.empty((N, C, L), np.float32)
    for c in range(N_CORES):
        n, half = c // 2, c % 2
        out[n][:, half * LH:(half + 1) * LH] = res.results[c]["out"]
    return out


# revision 24
# speedup vs baseline: 1.0107x; 1.0107x over previous
"""TRN2 Bass kernel for nn_AttentionBlock (N=4, C=256, L=4096, 4 heads, AGGR=4).

Sharding: 8 cores = (batch n, L-half). Core c handles n=c//2, query positions
l in [half*2048, (half+1)*2048). Each core computes k/v from the full
aggregated sequence of its batch (L2=1024) and produces the full output slice
out[n][:, l_half] -- no cross-core reduction needed.

The host hands each core x[n] with columns PERMUTED so the core's own query
half comes first (attention is permutation-invariant over key positions, and
the 4-wide pooling windows stay intact), so the query slice is a static
[:, 0:2048] view and is available as soon as the first DMA half lands.

Cost-model shape: every engine instruction costs (free-dim cols) x cycle_t;
PE matmuls cost (out free cols) x 0.417ns regardless of contraction width.
The kernel is PE-bound (~70us of matmul cols), so softmax exp -- the other
big consumer (64 tiles x 1024 cols) -- is split across three engines so none
exceeds PE: ACT runs real Exp; Pool and DVE run a Schraudolph bit-trick exp
(i32 = trunc(S*2^23*log2e*0.125 + bias), bitcast as f32 ~ exp(S/8) within
3%), writing int32 tiles the o-matmul consumes as float32r. f32->f32r
bitcast views avoid all weight/x re-typing copies.
"""

import numpy as np

N, C, L = 4, 256, 4096
HEAD_DIM = 64
H = C // HEAD_DIM          # 4 heads
AGGR = 4
L2 = L // AGGR             # 1024 aggregated positions
LH = L // 2                # 2048 query positions per core
BN_EPS = 1e-5
N_CORES = 8

# Schraudolph exp-trick constants, int16/bfloat16 variant (trunc/floor):
# i16 = trunc(S * 2^7*log2e/8 + (127*2^7 - bias)); i16 bits read as bf16
# give exp(S/8) within ~3%. Folds the 1/sqrt(E)=1/8 score scale.
TRICK_A = 128.0 * 1.4426950408889634 * 0.125
TRICK_B = float(127 << 7) - 366400.0 / 65536.0

_CACHE = {}


def _build_program():
    import concourse.bass as bass
    import concourse.bacc as bacc
    import concourse.tile as tile
    from concourse import mybir
    from contextlib import ExitStack

    dt = mybir.dt
    f32 = dt.float32
    f32r = dt.float32r
    bf16 = dt.bfloat16
    i16 = dt.int16
    AF = mybir.ActivationFunctionType
    Alu = mybir.AluOpType

    nc = bacc.Bacc("TRN2", debug=False, num_devices=N_CORES)

    xf_d = nc.dram_tensor("x_full", [C, L], f32, kind="ExternalInput")
    wqt_d = nc.dram_tensor("wqt", [C, C], f32, kind="ExternalInput")
    wkt_d = nc.dram_tensor("wkt", [C, C], f32, kind="ExternalInput")
    wvt_d = nc.dram_tensor("wvt", [C, C], f32, kind="ExternalInput")
    wot_d = nc.dram_tensor("wot", [C, C], f32, kind="ExternalInput")
    wat_d = nc.dram_tensor("wat", [C, C], f32, kind="ExternalInput")
    # rows: bq, bk, t(bn-folded xa bias), bo
    bp_d = nc.dram_tensor("biasp", [4, C], f32, kind="ExternalInput")
    bv_d = nc.dram_tensor("bv", [C], f32, kind="ExternalInput")
    out_d = nc.dram_tensor("out", [C, LH], f32, kind="ExternalOutput")

    # exp engine per (iter_index, mt): A=ACT real exp, D=DVE int-trick exp
    # (gpsimd can't read PSUM, so only ACT/DVE can consume the S tiles)
    EXP_ENG = {}
    for it in range(8):
        for mt in range(8):
            d = it < 7 and (mt == 7 or (mt == 6 and it < 6))
            EXP_ENG[(it, mt)] = 'D' if d else 'A'

    with tile.TileContext(nc) as tc, ExitStack() as ctx:
        pp = ctx.enter_context(tc.tile_pool(name="persist", bufs=1))
        scr_w = ctx.enter_context(tc.tile_pool(name="scr_w", bufs=1))
        scr_p = ctx.enter_context(tc.tile_pool(name="scr_p", bufs=3))
        at_pool = ctx.enter_context(tc.tile_pool(name="at", bufs=6))
        ai_pool = ctx.enter_context(tc.tile_pool(name="ai", bufs=5))
        oa_pool = ctx.enter_context(tc.tile_pool(name="oa", bufs=2))
        outp = ctx.enter_context(tc.tile_pool(name="outp", bufs=3))
        r_pool = ctx.enter_context(tc.tile_pool(name="rp", bufs=2))
        R_pool = ctx.enter_context(tc.tile_pool(name="Rp", bufs=3))

        ps_s = ctx.enter_context(tc.tile_pool(name="ps_s", bufs=3, space="PSUM"))
        ps_o = ctx.enter_context(tc.tile_pool(name="ps_o", bufs=2, space="PSUM"))

        # ---- persistent tiles ----
        xf = [pp.tile([128, L], f32, name=f"xf{ct}", tag=f"xf{ct}")
              for ct in range(2)]
        # query-half of x re-DMA'd as f32r for the q projection (the BIR
        # verifier requires f32r matmul inputs to be produced as f32r)
        xq_r = [pp.tile([128, LH], f32r, name=f"xqr{ct}", tag=f"xqr{ct}")
                for ct in range(2)]
        q_r = [pp.tile([128, LH], bf16, name=f"qr{ct}", tag=f"qr{ct}")
               for ct in range(2)]
        k_r = [pp.tile([128, L2], bf16, name=f"kr{ct}", tag=f"kr{ct}")
               for ct in range(2)]
        xa_r = [pp.tile([128, L2], f32r, name=f"xar{ct}", tag=f"xar{ct}")
                for ct in range(2)]
        p_r = [pp.tile([128, L2], f32r, name=f"pr{ct}", tag=f"pr{ct}")
               for ct in range(2)]
        # v'^T per m-tile: 4 heads x (64 cols + ones col), bf16
        v_r = [pp.tile([128, 4 * 65], bf16, name=f"vr{mt}", tag=f"vr{mt}")
               for mt in range(8)]
        bias_t = [pp.tile([128, 4], f32, name=f"bias{ct}", tag=f"bias{ct}")
                  for ct in range(2)]
        bvb = pp.tile([128, C], f32, name="bvb", tag="bvb")

        # ---- DMAs: x on the SP queue; weights/biases on the ACT queue so
        # they land in parallel with the 4MB of x ----
        for half in range(2):
            for sub in range(2):
                for ct in range(2):
                    c0 = half * 2048 + sub * 1024
                    nc.sync.dma_start(
                        xf[ct][:, c0:c0 + 1024],
                        xf_d.ap()[ct * 128:(ct + 1) * 128, c0:c0 + 1024])
        wt_dram = {"wqt": wqt_d, "wkt": wkt_d, "wvt": wvt_d, "wot": wot_d,
                   "wat": wat_d}
        w_f = {}

        def w_dma(wname):
            wf = scr_w.tile([128, 512], f32r, name=f"wf_{wname}",
                            tag=f"wf_{wname}")
            src = wt_dram[wname].ap().bitcast(f32r).rearrange(
                "(k p) o -> p k o", p=128)
            nc.scalar.dma_start(wf[:].rearrange("p (k o) -> p k o", k=2), src)
            w_f[wname] = wf

        def xq_dma(c0, cw):
            for ct in range(2):
                nc.scalar.dma_start(
                    xq_r[ct][:, c0:c0 + cw],
                    xf_d.ap().bitcast(f32r)[ct * 128:(ct + 1) * 128,
                                            c0:c0 + cw])

        # ACT-queue order tracks first-use: q_chunk(0) fires first (xq cols
        # 0:512 + wqt), then the pool->xa->k chain weights, then the rest.
        xq_dma(0, 512)
        for wname in ("wqt", "wat", "wkt", "wvt"):
            w_dma(wname)
        for ct in range(2):
            nc.scalar.dma_start(
                bias_t[ct][:], bp_d.ap().rearrange("b (k p) -> k p b", p=128)[ct])
        bv_f = r_pool.tile([1, C], f32, name="bv_f", tag="bv_f", bufs=1)
        nc.scalar.dma_start(bv_f[:], bv_d.ap().rearrange("(a o) -> a o", a=1))
        xq_dma(512, 1536)
        w_dma("wot")

        # ---- constants ----
        for mt in range(8):
            nc.gpsimd.memset(
                v_r[mt][:].rearrange("p (h e) -> p h e", e=65)[:, :, 64], 1.0)
        # pre-warm the ACT exp table during the idle prefix
        warm = scr_w.tile([1, 8], f32, name="warm", tag="warm")
        ones_f = scr_w.tile([1, 8], f32, name="ones_f", tag="ones_f")
        nc.gpsimd.memset(ones_f[:], 1.0)
        nc.scalar.activation(warm[:], ones_f[:], AF.Exp, scale=1.0)
        # bv broadcast to all partitions (for the v-drain fused bias)
        nc.gpsimd.partition_broadcast(bvb[:], bv_f[:], channels=128)

        def w_block(wname, cch, ct_out):
            # lhsT block [c_in 128, c_out 128] for chunk cch, out tile ct_out
            return w_f[wname][:, cch * 256 + ct_out * 128:
                              cch * 256 + ct_out * 128 + 128]

        # ---- pool quadrants: p = avg4 + max4 ----
        def pool_quadrant(mc, ct, eng, sub=None):
            c0, cw = mc * 2048, 2048
            s0, sw = mc * 512, 512
            if sub is not None:
                c0, cw = c0 + sub * 1024, 1024
                s0, sw = s0 + sub * 256, 256
            xv = xf[ct][:, c0:c0 + cw].rearrange("p (m g) -> p m g", g=4)
            a1 = scr_p.tile([128, 512], f32, name="pa1", tag="pa1")
            a2 = scr_p.tile([128, 512], f32, name="pa2", tag="pa2")
            m1 = scr_p.tile([128, 512], f32, name="pm1", tag="pm1")
            m2 = scr_p.tile([128, 512], f32, name="pm2", tag="pm2")
            eng.tensor_tensor(a1[:, 0:sw], xv[:, :, 0], xv[:, :, 1], Alu.add)
            eng.tensor_tensor(a2[:, 0:sw], xv[:, :, 2], xv[:, :, 3], Alu.add)
            eng.tensor_tensor(m1[:, 0:sw], xv[:, :, 0], xv[:, :, 1], Alu.max)
            eng.tensor_tensor(m2[:, 0:sw], xv[:, :, 2], xv[:, :, 3], Alu.max)
            eng.tensor_tensor(a1[:, 0:sw], a1[:, 0:sw], a2[:, 0:sw], Alu.add)
            eng.tensor_tensor(m1[:, 0:sw], m1[:, 0:sw], m2[:, 0:sw], Alu.max)
            eng.scalar_tensor_tensor(
                p_r[ct][:, s0:s0 + sw], a1[:, 0:sw], 0.25, m1[:, 0:sw],
                Alu.mult, Alu.add)

        # first quadrants in halves: start right after the first x chunks
        pool_quadrant(0, 0, nc.vector, sub=0)
        pool_quadrant(0, 0, nc.vector, sub=1)
        pool_quadrant(0, 1, nc.vector, sub=0)
        pool_quadrant(0, 1, nc.vector, sub=1)

        # ---- projection chunk helpers ----
        def proj_chunk(wname, src, dst, bias_col, nn2, eng):
            for ct_out in range(2):
                ps = ps_s.tile([128, 512], f32, name="ps_s", tag="ps_s")
                for cch in range(2):
                    nc.tensor.matmul(
                        ps[:], w_block(wname, cch, ct_out),
                        src[cch][:, nn2 * 512:(nn2 + 1) * 512],
                        start=(cch == 0), stop=(cch == 1))
                if eng is nc.scalar:
                    nc.scalar.add(dst[ct_out][:, nn2 * 512:(nn2 + 1) * 512],
                                  ps[:], bias_t[ct_out][:, bias_col:bias_col + 1])
                else:
                    eng.tensor_scalar(
                        dst[ct_out][:, nn2 * 512:(nn2 + 1) * 512], ps[:],
                        bias_t[ct_out][:, bias_col:bias_col + 1], None, Alu.add)

        def q_chunk(lcq, eng):
            for ct_out in range(2):
                ps = ps_s.tile([128, 512], f32, name="ps_s", tag="ps_s")
                for cch in range(2):
                    nc.tensor.matmul(
                        ps[:], w_block("wqt", cch, ct_out),
                        xq_r[cch][:, lcq * 512:(lcq + 1) * 512],
                        start=(cch == 0), stop=(cch == 1))
                if eng is nc.scalar:
                    nc.scalar.add(q_r[ct_out][:, lcq * 512:(lcq + 1) * 512],
                                  ps[:], bias_t[ct_out][:, 0:1])
                else:
                    eng.tensor_scalar(
                        q_r[ct_out][:, lcq * 512:(lcq + 1) * 512], ps[:],
                        bias_t[ct_out][:, 0:1], None, Alu.add)

        def v_block(mt, drain_eng, vpool=None):
            vpool = vpool or ps_o
            tag = "ps_o" if vpool is ps_o else "ps_s"
            pv = vpool.tile([128, C], f32, name="ps_v", tag=tag)
            for cch in range(2):
                nc.tensor.matmul(
                    pv[:], xa_r[cch][:, mt * 128:(mt + 1) * 128],
                    w_f["wvt"][:, cch * 256:(cch + 1) * 256],
                    start=(cch == 0), stop=(cch == 1))
            vv = v_r[mt][:].rearrange("p (h e) -> p h e", e=65)
            # fused +bv via the broadcast bias tile
            drain_eng.scalar_tensor_tensor(
                vv[:, :, 0:64], pv[:].rearrange("p (h e) -> p h e", e=64),
                1.0, bvb[:].rearrange("p (h e) -> p h e", e=64),
                Alu.mult, Alu.add)

        # ---- prefix: q lc0, xa/k chunk n0, rest of pool, v 0-3 ----
        q_chunk(0, nc.scalar)
        proj_chunk("wat", p_r, xa_r, 2, 0, nc.scalar)
        proj_chunk("wkt", xa_r, k_r, 1, 0, nc.scalar)
        pool_quadrant(1, 0, nc.vector)
        for mt in range(2):
            v_block(mt, nc.vector)
        pool_quadrant(1, 1, nc.vector)
        for mt in range(2, 4):
            v_block(mt, nc.vector)

        # ---- attention: o-matmuls lag exp by one m-tile; the previous
        # iteration's softmax-normalize and Wo conv are emitted inside the
        # next iteration's S/exp stream so they overlap it ----
        oa_tiles = {}

        def norm_prev(state):
            lc, hp, po = state
            oa = oa_tiles[lc]
            for h2 in range(2):
                r_t = r_pool.tile([1, 512], f32, name="r", tag="r")
                nc.vector.reciprocal(r_t[:], po[h2][64:65, :])
                R_t = R_pool.tile([64, 512], f32, name="R", tag="R")
                nc.gpsimd.partition_broadcast(R_t[:], r_t[:], channels=64)
                nc.vector.tensor_tensor(
                    oa[hp][h2 * 64:(h2 + 1) * 64, :], po[h2][0:64, :],
                    R_t[:], Alu.mult)

        def wo_prev(state):
            lc, hp, po = state
            if hp != 1:
                return
            oa = oa_tiles[lc]
            for ct_out in range(2):
                psW = ps_s.tile([128, 512], f32, name="ps_s", tag="ps_s")
                for cch in range(2):
                    nc.tensor.matmul(
                        psW[:], w_block("wot", cch, ct_out), oa[cch][:],
                        start=(cch == 0), stop=(cch == 1))
                out_t = outp.tile([128, 512], f32, name="out", tag="out")
                nc.scalar.add(out_t[:], psW[:], bias_t[ct_out][:, 3:4])
                nc.sync.dma_start(
                    out_d.ap()[ct_out * 128:(ct_out + 1) * 128,
                               lc * 512:(lc + 1) * 512], out_t[:])
            del oa_tiles[lc]

        # pending o-matmul FIFO: one pair popped per (S, exp) step, crossing
        # iteration boundaries so PE never waits on the last exp of an iter
        pending = []
        it_idx = [0]

        def emit_iter(lc, hp, prev_state, mid_hook=None):
            it = it_idx[0]
            it_idx[0] += 1
            if hp == 0:
                oa_tiles[lc] = [
                    oa_pool.tile([128, 512], f32r, name=f"oa{ct}",
                                 tag=f"oa{ct}") for ct in range(2)]
            po = [ps_o.tile([65, 512], f32, name="ps_o", tag="ps_o")
                  for _ in range(2)]

            def make_o(mt, at_ap):
                def emit():
                    for h2 in range(2):
                        h = 2 * hp + h2
                        nc.tensor.matmul(
                            po[h2][:], v_r[mt][:, h * 65:h * 65 + 65],
                            at_ap[:, h2 * 512:(h2 + 1) * 512],
                            start=(mt == 0), stop=(mt == 7))
                return emit

            for mt in range(8):
                if mt == 4 and mid_hook is not None:
                    mid_hook()
                ps = ps_s.tile([128, L2], f32, name="ps_s", tag="ps_s")
                for h2 in range(2):
                    nc.tensor.matmul(
                        ps[:, h2 * 512:(h2 + 1) * 512],
                        k_r[hp][h2 * 64:(h2 + 1) * 64, mt * 128:(mt + 1) * 128],
                        q_r[hp][h2 * 64:(h2 + 1) * 64, lc * 512:(lc + 1) * 512],
                        start=True, stop=True)
                eng = EXP_ENG[(it, mt)]
                if eng == 'A':
                    at = at_pool.tile([128, 1024], bf16, name="at", tag="at")
                    nc.scalar.activation(at[:], ps[:], AF.Exp, scale=0.125)
                    at_ap = at[:]
                else:
                    ai = ai_pool.tile([128, 1024], i16, name="ai", tag="ai")
                    nc.vector.tensor_scalar(ai[:], ps[:], TRICK_A, TRICK_B,
                                            Alu.mult, Alu.add)
                    at_ap = ai[:].bitcast(bf16)
                pending.append(make_o(mt, at_ap))
                # with the deeper o-FIFO, the previous iteration's last
                # o-matmul is popped during step mt1, so its normalize may
                # be emitted no earlier than mt2 (else it misses mt7)
                if mt == 2 and prev_state is not None:
                    norm_prev(prev_state)
                if mt == 5 and prev_state is not None:
                    wo_prev(prev_state)
                if len(pending) >= 3:
                    pending.pop(0)()
            return (lc, hp, po)

        # iteration (0,0) with the n1 projections + v 4-7 emitted mid-stream
        def mid():
            proj_chunk("wat", p_r, xa_r, 2, 1, nc.scalar)
            proj_chunk("wkt", xa_r, k_r, 1, 1, nc.scalar)
            for mt in range(4, 8):
                v_block(mt, nc.vector, vpool=ps_s)

        state = emit_iter(0, 0, None, mid_hook=mid)
        q_after = {(0, 1): 1, (1, 0): 2, (1, 1): 3}
        for lc, hp in [(0, 1), (1, 0), (1, 1), (2, 0), (2, 1), (3, 0), (3, 1)]:
            state = emit_iter(lc, hp, state)
            lcq = q_after.get((lc, hp))
            if lcq:
                q_chunk(lcq, nc.vector)
        while pending:
            pending.pop(0)()
        norm_prev(state)
        wo_prev(state)

    nc.compile()
    return nc


def _get_program():
    if "nc" not in _CACHE:
        _CACHE["nc"] = _build_program()
    return _CACHE["nc"]


def kernel(x, Wq, bq, Wk, bk, Wv, bv, Wo, bo, Wa,
           g1, b1, m1, v1, g2, b2, m2, v2):
    from concourse import bass_utils

    nc = _get_program()

    x = np.asarray(x, dtype=np.float32)
    # fold both eval-mode BNs into a per-channel affine: xa = s*(Wa@p) + t
    s1 = np.asarray(g1) / np.sqrt(np.asarray(v1) + BN_EPS)
    t1 = np.asarray(b1) - np.asarray(m1) * s1
    s2 = np.asarray(g2) / np.sqrt(np.asarray(v2) + BN_EPS)
    t2 = np.asarray(b2) - np.asarray(m2) * s2
    s = (s1 * s2).astype(np.float32)
    t = (t1 * s2 + t2).astype(np.float32)

    wat = (np.asarray(Wa) * s[:, None]).astype(np.float32).T.copy()
    wqt = np.asarray(Wq, dtype=np.float32).T.copy()
    wkt = np.asarray(Wk, dtype=np.float32).T.copy()
    wvt = np.asarray(Wv, dtype=np.float32).T.copy()
    wot = np.asarray(Wo, dtype=np.float32).T.copy()
    biasp = np.stack([np.asarray(bq), np.asarray(bk), t,
                      np.asarray(bo)]).astype(np.float32)
    bvv = np.asarray(bv, dtype=np.float32)

    shared = {"wqt": wqt, "wkt": wkt, "wvt": wvt, "wot": wot, "wat": wat,
              "biasp": biasp, "bv": bvv}
    in_maps = []
    for c in range(N_CORES):
        n, half = c // 2, c % 2
        m = dict(shared)
        xs = x[n]
        if half == 0:
            m["x_full"] = np.ascontiguousarray(xs)
        else:
            # core's own query half first; key order is irrelevant
            # (pool windows intact, attention permutation-invariant)
            m["x_full"] = np.concatenate([xs[:, LH:], xs[:, :LH]], axis=1)
        in_maps.append(m)

    res = bass_utils.run_bass_kernel_spmd(nc, in_maps,
                                          core_ids=list(range(N_CORES)))
    out = np.empty((N, C, L), np.float32)
    for c in range(N_CORES):
        n, half = c // 2, c % 2
        out[n][:, half * LH:(half + 1) * LH] = res.results[c]["out"]
    return out


# revision 32
# speedup vs baseline: 1.0309x; 1.0200x over previous
"""TRN2 Bass kernel for nn_AttentionBlock (N=4, C=256, L=4096, 4 heads, AGGR=4).

Sharding: 8 cores = (batch n, L-half). Core c handles n=c//2, query positions
l in [half*2048, (half+1)*2048). Each core computes k/v from the full
aggregated sequence of its batch (L2=1024) and produces the full output slice
out[n][:, l_half] -- no cross-core reduction needed.

The host hands each core x[n] with columns PERMUTED so the core's own query
half comes first (attention is permutation-invariant over key positions, and
the 4-wide pooling windows stay intact), so the query slice is a static
[:, 0:2048] view and is available as soon as the first DMA half lands.

Cost-model shape: every engine instruction costs (free-dim cols) x cycle_t;
PE matmuls cost (out free cols) x 0.417ns regardless of contraction width.
The kernel is PE-bound (~70us of matmul cols), so softmax exp -- the other
big consumer (64 tiles x 1024 cols) -- is split across three engines so none
exceeds PE: ACT runs real Exp; Pool and DVE run a Schraudolph bit-trick exp
(i32 = trunc(S*2^23*log2e*0.125 + bias), bitcast as f32 ~ exp(S/8) within
3%), writing int32 tiles the o-matmul consumes as float32r. f32->f32r
bitcast views avoid all weight/x re-typing copies.
"""

import numpy as np

N, C, L = 4, 256, 4096
HEAD_DIM = 64
H = C // HEAD_DIM          # 4 heads
AGGR = 4
L2 = L // AGGR             # 1024 aggregated positions
LH = L // 2                # 2048 query positions per core
BN_EPS = 1e-5
N_CORES = 8

# Schraudolph exp-trick constants, int16/bfloat16 variant (trunc/floor):
# i16 = trunc(S * 2^7*log2e/8 + (127*2^7 - bias)); i16 bits read as bf16
# give exp(S/8) within ~3%. Folds the 1/sqrt(E)=1/8 score scale.
TRICK_A = 128.0 * 1.4426950408889634 * 0.125
TRICK_B = float(127 << 7) - 366400.0 / 65536.0

_CACHE = {}


def _build_program():
    import concourse.bass as bass
    import concourse.bacc as bacc
    import concourse.tile as tile
    from concourse import mybir
    from contextlib import ExitStack

    dt = mybir.dt
    f32 = dt.float32
    f32r = dt.float32r
    bf16 = dt.bfloat16
    i16 = dt.int16
    AF = mybir.ActivationFunctionType
    Alu = mybir.AluOpType

    nc = bacc.Bacc("TRN2", debug=False, num_devices=N_CORES)

    xf_d = nc.dram_tensor("x_full", [C, L], f32, kind="ExternalInput")
    wqt_d = nc.dram_tensor("wqt", [C, C], f32, kind="ExternalInput")
    wkt_d = nc.dram_tensor("wkt", [C, C], f32, kind="ExternalInput")
    wvt_d = nc.dram_tensor("wvt", [C, C], f32, kind="ExternalInput")
    wot_d = nc.dram_tensor("wot", [C, C], f32, kind="ExternalInput")
    wat_d = nc.dram_tensor("wat", [C, C], f32, kind="ExternalInput")
    # rows: bq, bk, t(bn-folded xa bias), bo
    bp_d = nc.dram_tensor("biasp", [4, C], f32, kind="ExternalInput")
    bv_d = nc.dram_tensor("bv", [C], f32, kind="ExternalInput")
    out_d = nc.dram_tensor("out", [C, LH], f32, kind="ExternalOutput")

    # exp engine per (iter_index, mt): A=ACT real exp, D=DVE int-trick exp
    # (gpsimd can't read PSUM, so only ACT/DVE can consume the S tiles)
    # iter 0: all-ACT (DVE is busy streaming pool blocks); iter 7: all-ACT
    # (the tail normalize/wo chain runs on DVE)
    EXP_ENG = {}
    for it in range(8):
        for mt in range(8):
            d = 1 <= it <= 6 and mt >= 6
            EXP_ENG[(it, mt)] = 'D' if d else 'A'

    with tile.TileContext(nc) as tc, ExitStack() as ctx:
        pp = ctx.enter_context(tc.tile_pool(name="persist", bufs=1))
        scr_w = ctx.enter_context(tc.tile_pool(name="scr_w", bufs=1))
        scr_p = ctx.enter_context(tc.tile_pool(name="scr_p", bufs=3))
        at_pool = ctx.enter_context(tc.tile_pool(name="at", bufs=6))
        ai_pool = ctx.enter_context(tc.tile_pool(name="ai", bufs=5))
        oa_pool = ctx.enter_context(tc.tile_pool(name="oa", bufs=2))
        outp = ctx.enter_context(tc.tile_pool(name="outp", bufs=3))
        r_pool = ctx.enter_context(tc.tile_pool(name="rp", bufs=2))
        R_pool = ctx.enter_context(tc.tile_pool(name="Rp", bufs=3))

        ps_s = ctx.enter_context(tc.tile_pool(name="ps_s", bufs=3, space="PSUM"))
        ps_o = ctx.enter_context(tc.tile_pool(name="ps_o", bufs=2, space="PSUM"))

        # ---- persistent tiles ----
        # x lands as f32r (bitcast DMA) so the q matmul can read it directly;
        # the pool/element-wise consumers don't care about the f32r tag
        xf = [pp.tile([128, L], f32r, name=f"xf{ct}", tag=f"xf{ct}")
              for ct in range(2)]
        q_r = [pp.tile([128, LH], bf16, name=f"qr{ct}", tag=f"qr{ct}")
               for ct in range(2)]
        k_r = [pp.tile([128, L2], bf16, name=f"kr{ct}", tag=f"kr{ct}")
               for ct in range(2)]
        xa_r = [pp.tile([128, L2], f32r, name=f"xar{ct}", tag=f"xar{ct}")
                for ct in range(2)]
        p_r = [pp.tile([128, L2], f32r, name=f"pr{ct}", tag=f"pr{ct}")
               for ct in range(2)]
        # v'^T per m-tile: 4 heads x (64 cols + ones col), bf16
        v_r = [pp.tile([128, 4 * 65], bf16, name=f"vr{mt}", tag=f"vr{mt}")
               for mt in range(8)]
        bias_t = [pp.tile([128, 4], f32, name=f"bias{ct}", tag=f"bias{ct}")
                  for ct in range(2)]
        bvb = pp.tile([128, C], f32, name="bvb", tag="bvb")

        # ---- DMAs: the cost model serializes DMA transfers (~360 B/ns
        # aggregate), so everything goes on the SP queue in first-use order;
        # putting DMAs on the ACT queue head-of-line-blocks its sequencer.
        wt_dram = {"wqt": wqt_d, "wkt": wkt_d, "wvt": wvt_d, "wot": wot_d,
                   "wat": wat_d}
        w_f = {}

        def w_dma(wname):
            wf = scr_w.tile([128, 512], f32r, name=f"wf_{wname}",
                            tag=f"wf_{wname}")
            src = wt_dram[wname].ap().bitcast(f32r).rearrange(
                "(k p) o -> p k o", p=128)
            nc.sync.dma_start(wf[:].rearrange("p (k o) -> p k o", k=2), src)
            w_f[wname] = wf

        def x_dma(half):
            for sub in range(2):
                for ct in range(2):
                    c0 = half * 2048 + sub * 1024
                    nc.sync.dma_start(
                        xf[ct][:, c0:c0 + 1024],
                        xf_d.ap().bitcast(f32r)[ct * 128:(ct + 1) * 128,
                                                c0:c0 + 1024])

        w_dma("wqt")
        x_dma(0)
        w_dma("wat")
        w_dma("wkt")
        x_dma(1)
        w_dma("wvt")
        for ct in range(2):
            nc.sync.dma_start(
                bias_t[ct][:], bp_d.ap().rearrange("b (k p) -> k p b", p=128)[ct])
        bv_f = r_pool.tile([1, C], f32, name="bv_f", tag="bv_f", bufs=1)
        nc.sync.dma_start(bv_f[:], bv_d.ap().rearrange("(a o) -> a o", a=1))
        w_dma("wot")

        # ---- constants ----
        for mt in range(8):
            nc.gpsimd.memset(
                v_r[mt][:].rearrange("p (h e) -> p h e", e=65)[:, :, 64], 1.0)
        # pre-warm the ACT exp table during the idle prefix
        warm = scr_w.tile([1, 8], f32, name="warm", tag="warm")
        ones_f = scr_w.tile([1, 8], f32, name="ones_f", tag="ones_f")
        nc.gpsimd.memset(ones_f[:], 1.0)
        nc.scalar.activation(warm[:], ones_f[:], AF.Exp, scale=1.0)
        # bv broadcast to all partitions (for the v-drain fused bias)
        nc.gpsimd.partition_broadcast(bvb[:], bv_f[:], channels=128)

        def w_block(wname, cch, ct_out):
            # lhsT block [c_in 128, c_out 128] for chunk cch, out tile ct_out
            return w_f[wname][:, cch * 256 + ct_out * 128:
                              cch * 256 + ct_out * 128 + 128]

        # ---- pool quadrants: p = avg4 + max4 ----
        def pool_quadrant(mc, ct, eng, sub=None):
            c0, cw = mc * 2048, 2048
            s0, sw = mc * 512, 512
            if sub is not None:
                c0, cw = c0 + sub * 1024, 1024
                s0, sw = s0 + sub * 256, 256
            xv = xf[ct][:, c0:c0 + cw].rearrange("p (m g) -> p m g", g=4)
            a1 = scr_p.tile([128, 512], f32, name="pa1", tag="pa1")
            a2 = scr_p.tile([128, 512], f32, name="pa2", tag="pa2")
            m1 = scr_p.tile([128, 512], f32, name="pm1", tag="pm1")
            m2 = scr_p.tile([128, 512], f32, name="pm2", tag="pm2")
            eng.tensor_tensor(a1[:, 0:sw], xv[:, :, 0], xv[:, :, 1], Alu.add)
            eng.tensor_tensor(a2[:, 0:sw], xv[:, :, 2], xv[:, :, 3], Alu.add)
            eng.tensor_tensor(m1[:, 0:sw], xv[:, :, 0], xv[:, :, 1], Alu.max)
            eng.tensor_tensor(m2[:, 0:sw], xv[:, :, 2], xv[:, :, 3], Alu.max)
            eng.tensor_tensor(a1[:, 0:sw], a1[:, 0:sw], a2[:, 0:sw], Alu.add)
            eng.tensor_tensor(m1[:, 0:sw], m1[:, 0:sw], m2[:, 0:sw], Alu.max)
            eng.scalar_tensor_tensor(
                p_r[ct][:, s0:s0 + sw], a1[:, 0:sw], 0.25, m1[:, 0:sw],
                Alu.mult, Alu.add)

        # ---- projection chunk helpers ----
        def proj_chunk(wname, src, dst, bias_col, c0, cw, eng):
            for ct_out in range(2):
                ps = ps_s.tile([128, cw], f32, name="ps_s", tag="ps_s")
                for cch in range(2):
                    nc.tensor.matmul(
                        ps[:], w_block(wname, cch, ct_out),
                        src[cch][:, c0:c0 + cw],
                        start=(cch == 0), stop=(cch == 1))
                if eng is nc.scalar:
                    nc.scalar.add(dst[ct_out][:, c0:c0 + cw],
                                  ps[:], bias_t[ct_out][:, bias_col:bias_col + 1])
                else:
                    eng.tensor_scalar(
                        dst[ct_out][:, c0:c0 + cw], ps[:],
                        bias_t[ct_out][:, bias_col:bias_col + 1], None, Alu.add)

        def q_chunk(lcq, eng):
            for ct_out in range(2):
                ps = ps_s.tile([128, 512], f32, name="ps_s", tag="ps_s")
                for cch in range(2):
                    nc.tensor.matmul(
                        ps[:], w_block("wqt", cch, ct_out),
                        xf[cch][:, lcq * 512:(lcq + 1) * 512],
                        start=(cch == 0), stop=(cch == 1))
                if eng is nc.scalar:
                    nc.scalar.add(q_r[ct_out][:, lcq * 512:(lcq + 1) * 512],
                                  ps[:], bias_t[ct_out][:, 0:1])
                else:
                    eng.tensor_scalar(
                        q_r[ct_out][:, lcq * 512:(lcq + 1) * 512], ps[:],
                        bias_t[ct_out][:, 0:1], None, Alu.add)

        def v_block(mt, drain_eng, vpool=None):
            vpool = vpool or ps_o
            tag = "ps_o" if vpool is ps_o else "ps_s"
            pv = vpool.tile([128, C], f32, name="ps_v", tag=tag)
            for cch in range(2):
                nc.tensor.matmul(
                    pv[:], xa_r[cch][:, mt * 128:(mt + 1) * 128],
                    w_f["wvt"][:, cch * 256:(cch + 1) * 256],
                    start=(cch == 0), stop=(cch == 1))
            vv = v_r[mt][:].rearrange("p (h e) -> p h e", e=65)
            # fused +bv via the broadcast bias tile
            drain_eng.scalar_tensor_tensor(
                vv[:, :, 0:64], pv[:].rearrange("p (h e) -> p h e", e=64),
                1.0, bvb[:].rearrange("p (h e) -> p h e", e=64),
                Alu.mult, Alu.add)

        # ---- key-block groups: 256 keys each, gated on one x DMA pair.
        # pool both ct tiles -> xa block -> k block -> two v blocks ----
        def blkgrp(b, drain_eng):
            mc, sub = divmod(b, 2)
            pool_quadrant(mc, 0, nc.vector, sub=sub)
            pool_quadrant(mc, 1, nc.vector, sub=sub)
            proj_chunk("wat", p_r, xa_r, 2, b * 256, 256, drain_eng)
            proj_chunk("wkt", xa_r, k_r, 1, b * 256, 256, drain_eng)
            for mt in (2 * b, 2 * b + 1):
                v_block(mt, nc.vector, vpool=ps_s)

        # ---- prefix: q0/q1 + first two key-block groups (x half 0) ----
        q_chunk(0, nc.scalar)
        blkgrp(0, nc.scalar)
        q_chunk(1, nc.scalar)
        blkgrp(1, nc.scalar)
        q_chunk(2, nc.vector)
        q_chunk(3, nc.vector)

        # ---- attention: o-matmuls lag exp by one m-tile; the previous
        # iteration's softmax-normalize and Wo conv are emitted inside the
        # next iteration's S/exp stream so they overlap it ----
        oa_tiles = {}

        def norm_prev(state):
            lc, hp, po = state
            oa = oa_tiles[lc]
            for h2 in range(2):
                r_t = r_pool.tile([1, 512], f32, name="r", tag="r")
                nc.vector.reciprocal(r_t[:], po[h2][64:65, :])
                R_t = R_pool.tile([64, 512], f32, name="R", tag="R")
                nc.gpsimd.partition_broadcast(R_t[:], r_t[:], channels=64)
                nc.vector.tensor_tensor(
                    oa[hp][h2 * 64:(h2 + 1) * 64, :], po[h2][0:64, :],
                    R_t[:], Alu.mult)

        def wo_prev(state):
            lc, hp, po = state
            if hp != 1:
                return
            oa = oa_tiles[lc]
            for ct_out in range(2):
                psW = ps_s.tile([128, 512], f32, name="ps_s", tag="ps_s")
                for cch in range(2):
                    nc.tensor.matmul(
                        psW[:], w_block("wot", cch, ct_out), oa[cch][:],
                        start=(cch == 0), stop=(cch == 1))
                out_t = outp.tile([128, 512], f32, name="out", tag="out")
                nc.scalar.add(out_t[:], psW[:], bias_t[ct_out][:, 3:4])
                nc.sync.dma_start(
                    out_d.ap()[ct_out * 128:(ct_out + 1) * 128,
                               lc * 512:(lc + 1) * 512], out_t[:])
            del oa_tiles[lc]

        # pending o-matmul FIFO: one pair popped per (S, exp) step, crossing
        # iteration boundaries so PE never waits on the last exp of an iter
        pending = []
        it_idx = [0]

        def emit_iter(lc, hp, prev_state, hooks=None):
            it = it_idx[0]
            it_idx[0] += 1
            if hp == 0:
                oa_tiles[lc] = [
                    oa_pool.tile([128, 512], f32r, name=f"oa{ct}",
                                 tag=f"oa{ct}") for ct in range(2)]
            po = [ps_o.tile([65, 512], f32, name="ps_o", tag="ps_o")
                  for _ in range(2)]

            def make_o(mt, at_ap):
                def emit():
                    for h2 in range(2):
                        h = 2 * hp + h2
                        nc.tensor.matmul(
                            po[h2][:], v_r[mt][:, h * 65:h * 65 + 65],
                            at_ap[:, h2 * 512:(h2 + 1) * 512],
                            start=(mt == 0), stop=(mt == 7))
                return emit

            for mt in range(8):
                if hooks and mt in hooks:
                    hooks[mt]()
                ps = ps_s.tile([128, L2], f32, name="ps_s", tag="ps_s")
                for h2 in range(2):
                    nc.tensor.matmul(
                        ps[:, h2 * 512:(h2 + 1) * 512],
                        k_r[hp][h2 * 64:(h2 + 1) * 64, mt * 128:(mt + 1) * 128],
                        q_r[hp][h2 * 64:(h2 + 1) * 64, lc * 512:(lc + 1) * 512],
                        start=True, stop=True)
                eng = EXP_ENG[(it, mt)]
                if eng == 'A':
                    at = at_pool.tile([128, 1024], bf16, name="at", tag="at")
                    nc.scalar.activation(at[:], ps[:], AF.Exp, scale=0.125)
                    at_ap = at[:]
                else:
                    ai = ai_pool.tile([128, 1024], i16, name="ai", tag="ai")
                    nc.vector.tensor_scalar(ai[:], ps[:], TRICK_A, TRICK_B,
                                            Alu.mult, Alu.add)
                    at_ap = ai[:].bitcast(bf16)
                pending.append(make_o(mt, at_ap))
                # with the deeper o-FIFO, the previous iteration's last
                # o-matmul is popped during step mt1, so its normalize may
                # be emitted no earlier than mt2 (else it misses mt7)
                if mt == 2 and prev_state is not None:
                    norm_prev(prev_state)
                if mt == 5 and prev_state is not None:
                    wo_prev(prev_state)
                if len(pending) >= 3:
                    pending.pop(0)()
            return (lc, hp, po)

        # iteration (0,0): key-block groups 2/3 (x half 1) stream in mid-iter
        state = emit_iter(0, 0, None, hooks={
            3: lambda: blkgrp(2, nc.vector),
            5: lambda: blkgrp(3, nc.vector),
        })
        for lc, hp in [(0, 1), (1, 0), (1, 1), (2, 0), (2, 1), (3, 0), (3, 1)]:
            state = emit_iter(lc, hp, state)
        while pending:
            pending.pop(0)()
        norm_prev(state)
        wo_prev(state)

    nc.compile()
    return nc


def _get_program():
    if "nc" not in _CACHE:
        _CACHE["nc"] = _build_program()
    return _CACHE["nc"]


def kernel(x, Wq, bq, Wk, bk, Wv, bv, Wo, bo, Wa,
           g1, b1, m1, v1, g2, b2, m2, v2):
    from concourse import bass_utils

    nc = _get_program()

    x = np.asarray(x, dtype=np.float32)
    # fold both eval-mode BNs into a per-channel affine: xa = s*(Wa@p) + t
    s1 = np.asarray(g1) / np.sqrt(np.asarray(v1) + BN_EPS)
    t1 = np.asarray(b1) - np.asarray(m1) * s1
    s2 = np.asarray(g2) / np.sqrt(np.asarray(v2) + BN_EPS)
    t2 = np.asarray(b2) - np.asarray(m2) * s2
    s = (s1 * s2).astype(np.float32)
    t = (t1 * s2 + t2).astype(np.float32)

    wat = (np.asarray(Wa) * s[:, None]).astype(np.float32).T.copy()
    wqt = np.asarray(Wq, dtype=np.float32).T.copy()
    wkt = np.asarray(Wk, dtype=np.float32).T.copy()
    wvt = np.asarray(Wv, dtype=np.float32).T.copy()
    wot = np.asarray(Wo, dtype=np.float32).T.copy()
    biasp = np.stack([np.asarray(bq), np.asarray(bk), t,
                      np.asarray(bo)]).astype(np.float32)
    bvv = np.asarray(bv, dtype=np.float32)

    shared = {"wqt": wqt, "wkt": wkt, "wvt": wvt, "wot": wot, "wat": wat,
              "biasp": biasp, "bv": bvv}
    in_maps = []
    for c in range(N_CORES):
        n, half = c // 2, c % 2
        m = dict(shared)
        xs = x[n]
        if half == 0:
            m["x_full"] = np.ascontiguousarray(xs)
        else:
            # core's own query half first; key order is irrelevant
            # (pool windows intact, attention permutation-invariant)
            m["x_full"] = np.concatenate([xs[:, LH:], xs[:, :LH]], axis=1)
        in_maps.append(m)

    res = bass_utils.run_bass_kernel_spmd(nc, in_maps,
                                          core_ids=list(range(N_CORES)))
    out = np.empty((N, C, L), np.float32)
    for c in range(N_CORES):
        n, half = c // 2, c % 2
        out[n][:, half * LH:(half + 1) * LH] = res.results[c]["out"]
    return out


# revision 35
# speedup vs baseline: 1.0394x; 1.0082x over previous
"""TRN2 Bass kernel for nn_AttentionBlock (N=4, C=256, L=4096, 4 heads, AGGR=4).

Sharding: 8 cores = (batch n, L-half). Core c handles n=c//2, query positions
l in [half*2048, (half+1)*2048). Each core computes k/v from the full
aggregated sequence of its batch (L2=1024) and produces the full output slice
out[n][:, l_half] -- no cross-core reduction needed.

The host hands each core x[n] with columns PERMUTED so the core's own query
half comes first (attention is permutation-invariant over key positions, and
the 4-wide pooling windows stay intact), so the query slice is a static
[:, 0:2048] view and is available as soon as the first DMA half lands.

Cost-model shape: every engine instruction costs (free-dim cols) x cycle_t;
PE matmuls cost (out free cols) x 0.417ns regardless of contraction width.
The kernel is PE-bound (~70us of matmul cols), so softmax exp -- the other
big consumer (64 tiles x 1024 cols) -- is split across three engines so none
exceeds PE: ACT runs real Exp; Pool and DVE run a Schraudolph bit-trick exp
(i32 = trunc(S*2^23*log2e*0.125 + bias), bitcast as f32 ~ exp(S/8) within
3%), writing int32 tiles the o-matmul consumes as float32r. f32->f32r
bitcast views avoid all weight/x re-typing copies.
"""

import numpy as np

N, C, L = 4, 256, 4096
HEAD_DIM = 64
H = C // HEAD_DIM          # 4 heads
AGGR = 4
L2 = L // AGGR             # 1024 aggregated positions
LH = L // 2                # 2048 query positions per core
BN_EPS = 1e-5
N_CORES = 8

# Schraudolph exp-trick constants, int16/bfloat16 variant (trunc/floor):
# i16 = trunc(S * 2^7*log2e/8 + (127*2^7 - bias)); i16 bits read as bf16
# give exp(S/8) within ~3%. Folds the 1/sqrt(E)=1/8 score scale.
TRICK_A = 128.0 * 1.4426950408889634 * 0.125
TRICK_B = float(127 << 7) - 366400.0 / 65536.0

_CACHE = {}


def _build_program():
    import concourse.bass as bass
    import concourse.bacc as bacc
    import concourse.tile as tile
    from concourse import mybir
    from contextlib import ExitStack

    dt = mybir.dt
    f32 = dt.float32
    f32r = dt.float32r
    bf16 = dt.bfloat16
    i16 = dt.int16
    AF = mybir.ActivationFunctionType
    Alu = mybir.AluOpType

    nc = bacc.Bacc("TRN2", debug=False, num_devices=N_CORES)

    xf_d = nc.dram_tensor("x_full", [C, L], f32, kind="ExternalInput")
    wqt_d = nc.dram_tensor("wqt", [C, C], f32, kind="ExternalInput")
    wkt_d = nc.dram_tensor("wkt", [C, C], f32, kind="ExternalInput")
    wvt_d = nc.dram_tensor("wvt", [C, C], f32, kind="ExternalInput")
    wot_d = nc.dram_tensor("wot", [C, C], f32, kind="ExternalInput")
    wat_d = nc.dram_tensor("wat", [C, C], f32, kind="ExternalInput")
    # rows: bq, bk, t(bn-folded xa bias), bo
    bp_d = nc.dram_tensor("biasp", [4, C], f32, kind="ExternalInput")
    bv_d = nc.dram_tensor("bv", [C], f32, kind="ExternalInput")
    out_d = nc.dram_tensor("out", [C, LH], f32, kind="ExternalOutput")

    # exp engine per (iter_index, mt): A=ACT real exp, D=DVE int-trick exp
    # (gpsimd can't read PSUM, so only ACT/DVE can consume the S tiles)
    # iter 0: all-ACT (DVE is busy streaming pool blocks); iter 7: all-ACT
    # (the tail normalize/wo chain runs on DVE)
    EXP_ENG = {}
    for it in range(8):
        for mt in range(8):
            d = 1 <= it <= 6 and mt >= 6
            EXP_ENG[(it, mt)] = 'D' if d else 'A'

    with tile.TileContext(nc) as tc, ExitStack() as ctx:
        pp = ctx.enter_context(tc.tile_pool(name="persist", bufs=1))
        scr_w = ctx.enter_context(tc.tile_pool(name="scr_w", bufs=1))
        scr_p = ctx.enter_context(tc.tile_pool(name="scr_p", bufs=3))
        at_pool = ctx.enter_context(tc.tile_pool(name="at", bufs=6))
        ai_pool = ctx.enter_context(tc.tile_pool(name="ai", bufs=5))
        oa_pool = ctx.enter_context(tc.tile_pool(name="oa", bufs=2))
        outp = ctx.enter_context(tc.tile_pool(name="outp", bufs=3))
        r_pool = ctx.enter_context(tc.tile_pool(name="rp", bufs=2))
        R_pool = ctx.enter_context(tc.tile_pool(name="Rp", bufs=3))

        ps_s = ctx.enter_context(tc.tile_pool(name="ps_s", bufs=3, space="PSUM"))
        ps_o = ctx.enter_context(tc.tile_pool(name="ps_o", bufs=2, space="PSUM"))

        # ---- persistent tiles ----
        # x lands as f32r (bitcast DMA) so the q matmul can read it directly;
        # the pool/element-wise consumers don't care about the f32r tag
        xf = [pp.tile([128, L], f32r, name=f"xf{ct}", tag=f"xf{ct}")
              for ct in range(2)]
        q_r = [pp.tile([128, LH], bf16, name=f"qr{ct}", tag=f"qr{ct}")
               for ct in range(2)]
        k_r = [pp.tile([128, L2], bf16, name=f"kr{ct}", tag=f"kr{ct}")
               for ct in range(2)]
        xa_r = [pp.tile([128, L2], f32r, name=f"xar{ct}", tag=f"xar{ct}")
                for ct in range(2)]
        p_r = [pp.tile([128, L2], f32r, name=f"pr{ct}", tag=f"pr{ct}")
               for ct in range(2)]
        # v'^T per m-tile: 4 heads x (64 cols + ones col), bf16
        v_r = [pp.tile([128, 4 * 65], bf16, name=f"vr{mt}", tag=f"vr{mt}")
               for mt in range(8)]
        bias_t = [pp.tile([128, 4], f32, name=f"bias{ct}", tag=f"bias{ct}")
                  for ct in range(2)]
        bvb = pp.tile([128, C], f32, name="bvb", tag="bvb")

        # ---- DMAs: the cost model serializes DMA transfers (~360 B/ns
        # aggregate), so everything goes on the SP queue in first-use order;
        # putting DMAs on the ACT queue head-of-line-blocks its sequencer.
        wt_dram = {"wqt": wqt_d, "wkt": wkt_d, "wvt": wvt_d, "wot": wot_d,
                   "wat": wat_d}
        w_f = {}

        def w_dma(wname):
            wf = scr_w.tile([128, 512], f32r, name=f"wf_{wname}",
                            tag=f"wf_{wname}")
            src = wt_dram[wname].ap().bitcast(f32r).rearrange(
                "(k p) o -> p k o", p=128)
            nc.sync.dma_start(wf[:].rearrange("p (k o) -> p k o", k=2), src)
            w_f[wname] = wf

        def x_dma(half, sub):
            for ct in range(2):
                c0 = half * 2048 + sub * 1024
                nc.sync.dma_start(
                    xf[ct][:, c0:c0 + 1024],
                    xf_d.ap().bitcast(f32r)[ct * 128:(ct + 1) * 128,
                                            c0:c0 + 1024])

        for ct in range(2):
            nc.sync.dma_start(
                bias_t[ct][:], bp_d.ap().rearrange("b (k p) -> k p b", p=128)[ct])
        bv_f = r_pool.tile([1, C], f32, name="bv_f", tag="bv_f", bufs=1)
        nc.sync.dma_start(bv_f[:], bv_d.ap().rearrange("(a o) -> a o", a=1))
        w_dma("wqt")
        x_dma(0, 0)   # cols 0:1024, both ct tiles
        w_dma("wat")
        w_dma("wkt")
        w_dma("wvt")
        x_dma(0, 1)
        x_dma(1, 0)
        x_dma(1, 1)
        w_dma("wot")

        # ---- constants ----
        for mt in range(8):
            nc.gpsimd.memset(
                v_r[mt][:].rearrange("p (h e) -> p h e", e=65)[:, :, 64], 1.0)
        # pre-warm the ACT exp table during the idle prefix
        warm = scr_w.tile([1, 8], f32, name="warm", tag="warm")
        ones_f = scr_w.tile([1, 8], f32, name="ones_f", tag="ones_f")
        nc.gpsimd.memset(ones_f[:], 1.0)
        nc.scalar.activation(warm[:], ones_f[:], AF.Exp, scale=1.0)
        # bv broadcast to all partitions (for the v-drain fused bias)
        nc.gpsimd.partition_broadcast(bvb[:], bv_f[:], channels=128)

        def w_block(wname, cch, ct_out):
            # lhsT block [c_in 128, c_out 128] for chunk cch, out tile ct_out
            return w_f[wname][:, cch * 256 + ct_out * 128:
                              cch * 256 + ct_out * 128 + 128]

        # ---- pool quadrants: p = avg4 + max4 ----
        def pool_quadrant(mc, ct, eng, sub=None):
            c0, cw = mc * 2048, 2048
            s0, sw = mc * 512, 512
            if sub is not None:
                c0, cw = c0 + sub * 1024, 1024
                s0, sw = s0 + sub * 256, 256
            xv = xf[ct][:, c0:c0 + cw].rearrange("p (m g) -> p m g", g=4)
            a1 = scr_p.tile([128, 512], f32, name="pa1", tag="pa1")
            a2 = scr_p.tile([128, 512], f32, name="pa2", tag="pa2")
            m1 = scr_p.tile([128, 512], f32, name="pm1", tag="pm1")
            m2 = scr_p.tile([128, 512], f32, name="pm2", tag="pm2")
            eng.tensor_tensor(a1[:, 0:sw], xv[:, :, 0], xv[:, :, 1], Alu.add)
            eng.tensor_tensor(a2[:, 0:sw], xv[:, :, 2], xv[:, :, 3], Alu.add)
            eng.tensor_tensor(m1[:, 0:sw], xv[:, :, 0], xv[:, :, 1], Alu.max)
            eng.tensor_tensor(m2[:, 0:sw], xv[:, :, 2], xv[:, :, 3], Alu.max)
            eng.tensor_tensor(a1[:, 0:sw], a1[:, 0:sw], a2[:, 0:sw], Alu.add)
            eng.tensor_tensor(m1[:, 0:sw], m1[:, 0:sw], m2[:, 0:sw], Alu.max)
            eng.scalar_tensor_tensor(
                p_r[ct][:, s0:s0 + sw], a1[:, 0:sw], 0.25, m1[:, 0:sw],
                Alu.mult, Alu.add)

        # ---- projection chunk helpers ----
        def proj_chunk(wname, src, dst, bias_col, c0, cw, eng):
            for ct_out in range(2):
                ps = ps_s.tile([128, cw], f32, name="ps_s", tag="ps_s")
                for cch in range(2):
                    nc.tensor.matmul(
                        ps[:], w_block(wname, cch, ct_out),
                        src[cch][:, c0:c0 + cw],
                        start=(cch == 0), stop=(cch == 1))
                if eng is nc.scalar:
                    nc.scalar.add(dst[ct_out][:, c0:c0 + cw],
                                  ps[:], bias_t[ct_out][:, bias_col:bias_col + 1])
                else:
                    eng.tensor_scalar(
                        dst[ct_out][:, c0:c0 + cw], ps[:],
                        bias_t[ct_out][:, bias_col:bias_col + 1], None, Alu.add)

        def q_chunk(lcq, eng):
            for ct_out in range(2):
                ps = ps_s.tile([128, 512], f32, name="ps_s", tag="ps_s")
                for cch in range(2):
                    nc.tensor.matmul(
                        ps[:], w_block("wqt", cch, ct_out),
                        xf[cch][:, lcq * 512:(lcq + 1) * 512],
                        start=(cch == 0), stop=(cch == 1))
                if eng is nc.scalar:
                    nc.scalar.add(q_r[ct_out][:, lcq * 512:(lcq + 1) * 512],
                                  ps[:], bias_t[ct_out][:, 0:1])
                else:
                    eng.tensor_scalar(
                        q_r[ct_out][:, lcq * 512:(lcq + 1) * 512], ps[:],
                        bias_t[ct_out][:, 0:1], None, Alu.add)

        def v_block(mt, drain_eng, vpool=None):
            vpool = vpool or ps_o
            tag = "ps_o" if vpool is ps_o else "ps_s"
            pv = vpool.tile([128, C], f32, name="ps_v", tag=tag)
            for cch in range(2):
                nc.tensor.matmul(
                    pv[:], xa_r[cch][:, mt * 128:(mt + 1) * 128],
                    w_f["wvt"][:, cch * 256:(cch + 1) * 256],
                    start=(cch == 0), stop=(cch == 1))
            vv = v_r[mt][:].rearrange("p (h e) -> p h e", e=65)
            # fused +bv via the broadcast bias tile
            drain_eng.scalar_tensor_tensor(
                vv[:, :, 0:64], pv[:].rearrange("p (h e) -> p h e", e=64),
                1.0, bvb[:].rearrange("p (h e) -> p h e", e=64),
                Alu.mult, Alu.add)

        # ---- key-block groups: 256 keys each, gated on one x DMA pair.
        # pool both ct tiles -> xa block -> k block -> two v blocks ----
        def blkgrp(b, drain_eng):
            mc, sub = divmod(b, 2)
            pool_quadrant(mc, 0, nc.vector, sub=sub)
            pool_quadrant(mc, 1, nc.vector, sub=sub)
            proj_chunk("wat", p_r, xa_r, 2, b * 256, 256, drain_eng)
            proj_chunk("wkt", xa_r, k_r, 1, b * 256, 256, drain_eng)
            for mt in (2 * b, 2 * b + 1):
                v_block(mt, nc.vector, vpool=ps_s)

        # ---- prefix: q0/q1 + first two key-block groups (x half 0) ----
        q_chunk(0, nc.scalar)
        blkgrp(0, nc.scalar)
        q_chunk(1, nc.scalar)
        blkgrp(1, nc.scalar)
        q_chunk(2, nc.vector)
        q_chunk(3, nc.vector)

        # ---- attention: o-matmuls lag exp by one m-tile; the previous
        # iteration's softmax-normalize and Wo conv are emitted inside the
        # next iteration's S/exp stream so they overlap it ----
        oa_tiles = {}

        def norm_prev(state):
            lc, hp, po = state
            oa = oa_tiles[lc]
            for h2 in range(2):
                r_t = r_pool.tile([1, 512], f32, name="r", tag="r")
                nc.vector.reciprocal(r_t[:], po[h2][64:65, :])
                R_t = R_pool.tile([64, 512], f32, name="R", tag="R")
                nc.gpsimd.partition_broadcast(R_t[:], r_t[:], channels=64)
                nc.vector.tensor_tensor(
                    oa[hp][h2 * 64:(h2 + 1) * 64, :], po[h2][0:64, :],
                    R_t[:], Alu.mult)

        def wo_prev(state):
            lc, hp, po = state
            if hp != 1:
                return
            oa = oa_tiles[lc]
            for ct_out in range(2):
                psW = ps_s.tile([128, 512], f32, name="ps_s", tag="ps_s")
                for cch in range(2):
                    nc.tensor.matmul(
                        psW[:], w_block("wot", cch, ct_out), oa[cch][:],
                        start=(cch == 0), stop=(cch == 1))
                out_t = outp.tile([128, 512], f32, name="out", tag="out")
                nc.scalar.add(out_t[:], psW[:], bias_t[ct_out][:, 3:4])
                nc.sync.dma_start(
                    out_d.ap()[ct_out * 128:(ct_out + 1) * 128,
                               lc * 512:(lc + 1) * 512], out_t[:])
            del oa_tiles[lc]

        # pending o-matmul FIFO: one pair popped per (S, exp) step, crossing
        # iteration boundaries so PE never waits on the last exp of an iter
        pending = []
        it_idx = [0]

        def emit_iter(lc, hp, prev_state, hooks=None):
            it = it_idx[0]
            it_idx[0] += 1
            if hp == 0:
                oa_tiles[lc] = [
                    oa_pool.tile([128, 512], f32r, name=f"oa{ct}",
                                 tag=f"oa{ct}") for ct in range(2)]
            po = [ps_o.tile([65, 512], f32, name="ps_o", tag="ps_o")
                  for _ in range(2)]

            def make_o(mt, at_ap):
                def emit():
                    for h2 in range(2):
                        h = 2 * hp + h2
                        nc.tensor.matmul(
                            po[h2][:], v_r[mt][:, h * 65:h * 65 + 65],
                            at_ap[:, h2 * 512:(h2 + 1) * 512],
                            start=(mt == 0), stop=(mt == 7))
                return emit

            for mt in range(8):
                if hooks and mt in hooks:
                    hooks[mt]()
                ps = ps_s.tile([128, L2], f32, name="ps_s", tag="ps_s")
                for h2 in range(2):
                    nc.tensor.matmul(
                        ps[:, h2 * 512:(h2 + 1) * 512],
                        k_r[hp][h2 * 64:(h2 + 1) * 64, mt * 128:(mt + 1) * 128],
                        q_r[hp][h2 * 64:(h2 + 1) * 64, lc * 512:(lc + 1) * 512],
                        start=True, stop=True)
                eng = EXP_ENG[(it, mt)]
                if eng == 'A':
                    at = at_pool.tile([128, 1024], bf16, name="at", tag="at")
                    nc.scalar.activation(at[:], ps[:], AF.Exp, scale=0.125)
                    at_ap = at[:]
                else:
                    ai = ai_pool.tile([128, 1024], i16, name="ai", tag="ai")
                    nc.vector.tensor_scalar(ai[:], ps[:], TRICK_A, TRICK_B,
                                            Alu.mult, Alu.add)
                    at_ap = ai[:].bitcast(bf16)
                pending.append(make_o(mt, at_ap))
                # with the deeper o-FIFO, the previous iteration's last
                # o-matmul is popped during step mt1, so its normalize may
                # be emitted no earlier than mt2 (else it misses mt7)
                if mt == 2 and prev_state is not None:
                    norm_prev(prev_state)
                if mt == 5 and prev_state is not None:
                    wo_prev(prev_state)
                if len(pending) >= 3:
                    pending.pop(0)()
            return (lc, hp, po)

        # iteration (0,0): key-block groups 2/3 (x half 1) stream in mid-iter
        state = emit_iter(0, 0, None, hooks={
            3: lambda: blkgrp(2, nc.vector),
            5: lambda: blkgrp(3, nc.vector),
        })
        for lc, hp in [(0, 1), (1, 0), (1, 1), (2, 0), (2, 1), (3, 0), (3, 1)]:
            state = emit_iter(lc, hp, state)
        while pending:
            pending.pop(0)()
        norm_prev(state)
        wo_prev(state)

    nc.compile()
    return nc


def _get_program():
    if "nc" not in _CACHE:
        _CACHE["nc"] = _build_program()
    return _CACHE["nc"]


def kernel(x, Wq, bq, Wk, bk, Wv, bv, Wo, bo, Wa,
           g1, b1, m1, v1, g2, b2, m2, v2):
    from concourse import bass_utils

    nc = _get_program()

    x = np.asarray(x, dtype=np.float32)
    # fold both eval-mode BNs into a per-channel affine: xa = s*(Wa@p) + t
    s1 = np.asarray(g1) / np.sqrt(np.asarray(v1) + BN_EPS)
    t1 = np.asarray(b1) - np.asarray(m1) * s1
    s2 = np.asarray(g2) / np.sqrt(np.asarray(v2) + BN_EPS)
    t2 = np.asarray(b2) - np.asarray(m2) * s2
    s = (s1 * s2).astype(np.float32)
    t = (t1 * s2 + t2).astype(np.float32)

    wat = (np.asarray(Wa) * s[:, None]).astype(np.float32).T.copy()
    wqt = np.asarray(Wq, dtype=np.float32).T.copy()
    wkt = np.asarray(Wk, dtype=np.float32).T.copy()
    wvt = np.asarray(Wv, dtype=np.float32).T.copy()
    wot = np.asarray(Wo, dtype=np.float32).T.copy()
    biasp = np.stack([np.asarray(bq), np.asarray(bk), t,
                      np.asarray(bo)]).astype(np.float32)
    bvv = np.asarray(bv, dtype=np.float32)

    shared = {"wqt": wqt, "wkt": wkt, "wvt": wvt, "wot": wot, "wat": wat,
              "biasp": biasp, "bv": bvv}
    in_maps = []
    for c in range(N_CORES):
        n, half = c // 2, c % 2
        m = dict(shared)
        xs = x[n]
        if half == 0:
            m["x_full"] = np.ascontiguousarray(xs)
        else:
            # core's own query half first; key order is irrelevant
            # (pool windows intact, attention permutation-invariant)
            m["x_full"] = np.concatenate([xs[:, LH:], xs[:, :LH]], axis=1)
        in_maps.append(m)

    res = bass_utils.run_bass_kernel_spmd(nc, in_maps,
                                          core_ids=list(range(N_CORES)))
    out = np.empty((N, C, L), np.float32)
    for c in range(N_CORES):
        n, half = c // 2, c % 2
        out[n][:, half * LH:(half + 1) * LH] = res.results[c]["out"]
    return out
